# revision 67
# baseline (speedup 1.0000x reference)
"""BoeNet greedy BFS rollout — Trainium2 Bass kernel (8 NeuronCores), v2.

Fully data-parallel: each core owns 512 of the 4096 flattened positions and
computes the full vocab row block for them.  No collectives.

v2 vs v1 (322us baseline):
  * Host ships the gathered embedding rows already TRANSPOSED (hembT
    [E, pos]), both f32 (for gates) and bf16 (for values) — no on-device
    transposes, and phase A starts as soon as the first 256KB chunk lands.
  * All 7 tree gates are folded to embedding space on the host
    (v = Wp Wc... wg), so ONE accumulated matmul (stationary [128,8])
    produces every gate z as rows [8, 512].  Thresholding is one DVE
    tensor_scalar; the ancestor-AND + broadcast to [128,512] is done with
    tiny K=7 matmuls against 0/1 selector columns followed by a compare.
  * h0/n1 values in bf16 (gates no longer read them), halving weight DMA
    and enabling 16-bit DVE throughput for the mask products.
  * Tiny/critical DMAs are issued first; Wout window prefetch is deferred.
  * bout is added on the host: phase-B drains become pure psum->bf16
    copies, alternated between DVE and Act.  16MB of bias-broadcast DMA
    traffic disappears.
  * Phase-B window 0 runs hc-outer so its first matmuls start as soon as
    pooT[0] lands, overlapping the agg tail.
"""
import sys

for _p in ('/opt/trn_rl_repo', '/opt/pypackages'):
    if _p not in sys.path:
        sys.path.insert(0, _p)

import numpy as np

B, S, V, E, H = 8, 512, 32000, 512, 512
NPOS = B * S              # 4096 flattened positions
NCORES = 8
PC_POS = NPOS // NCORES   # 512 positions per core
MAX_DEPTH = 3
DEPTH_EMBED_SCALE = 0.01
SIB_SCALE = 1.0 / np.sqrt(H)

VW = 2000                 # vocab window (4 strips of 500)
NWIN = V // VW            # 16 windows
NSTR = 4                  # 500-col strips per window

# masks: e0, e1L, e1R, e2LL, e2LR, e2RL, e2RR
MASK_SEL = [(0,), (0, 1), (0, 2), (0, 1, 3), (0, 1, 4), (0, 2, 5), (0, 2, 6)]

_CACHE = {}
DEBUG_DUMP = False


def _build():
    import concourse.bass as bass
    import concourse.bacc as bacc
    import concourse.tile as tile
    import concourse.mybir as mybir
    from concourse.masks import make_identity
    from contextlib import ExitStack

    F32 = mybir.dt.float32
    F32R = mybir.dt.float32r
    BF16 = mybir.dt.bfloat16
    AF = mybir.ActivationFunctionType
    OP = mybir.AluOpType

    nc = bacc.Bacc("TRN2", target_bir_lowering=False, debug=False,
                   num_devices=NCORES)

    # --- dram inputs (order here is irrelevant; DMA issue order matters) ---
    wzb_d = nc.dram_tensor("wzb", [128, 32], BF16, kind="ExternalInput")
    wzr_d = nc.dram_tensor("wzr", [128, 32], BF16, kind="ExternalInput")
    thr7_d = nc.dram_tensor("thr7", [8, 1], F32, kind="ExternalInput")
    selb_d = nc.dram_tensor("selb", [7, 7 * 128], BF16, kind="ExternalInput")
    sel7_d = nc.dram_tensor("sel7", [7, 8], BF16, kind="ExternalInput")
    cthr_d = nc.dram_tensor("cthr", [8, 1], F32, kind="ExternalInput")
    bw7_d = nc.dram_tensor("bw7", [7, H], BF16, kind="ExternalInput")
    cols_d = nc.dram_tensor("cols", [128, 12], F32, kind="ExternalInput")
    hembtb_d = nc.dram_tensor("hembtb", [128, 4 * 512], BF16,
                              kind="ExternalInput")
    hembrb_d = nc.dram_tensor("hembrb", [128, 4 * 512], BF16,
                              kind="ExternalInput")
    wpb_d = nc.dram_tensor("wpb", [E, H], BF16, kind="ExternalInput")
    wcb_d = nc.dram_tensor("wcb", [H, 2 * H], BF16, kind="ExternalInput")
    wcsb_d = nc.dram_tensor("wcsb", [H, H], BF16, kind="ExternalInput")
    wab_d = nc.dram_tensor("wab", [H, H], BF16, kind="ExternalInput")
    wbb_d = nc.dram_tensor("wbb", [H, H], BF16, kind="ExternalInput")
    woutb_d = nc.dram_tensor("woutb", [H, V], BF16, kind="ExternalInput")
    logt_d = nc.dram_tensor("logt", [PC_POS, V], BF16, kind="ExternalOutput")
    if DEBUG_DUMP:
        dzr_d = nc.dram_tensor("dzr", [8, 512], BF16, kind="ExternalOutput")
        dmask_d = nc.dram_tensor("dmask", [128, 7 * 512], BF16,
                                 kind="ExternalOutput")
        dh0_d = nc.dram_tensor("dh0", [128, 4 * 512], BF16,
                               kind="ExternalOutput")
        dpoo_d = nc.dram_tensor("dpoo", [128, 4 * 512], BF16,
                                kind="ExternalOutput")
        dn1_d = nc.dram_tensor("dn1", [128, 8 * 512], BF16,
                               kind="ExternalOutput")
        dprod_d = nc.dram_tensor("dprod", [128, 12 * 512], BF16,
                                 kind="ExternalOutput")
        drow_d = nc.dram_tensor("drow", [128, 4 * 512], mybir.dt.float32,
                                kind="ExternalOutput")
        dagg_d = nc.dram_tensor("dagg", [128, 4 * 512], mybir.dt.float32,
                                kind="ExternalOutput")

    with tile.TileContext(nc) as tc, ExitStack() as ctx:
        const = ctx.enter_context(tc.tile_pool(name="const", bufs=1))
        wpool = ctx.enter_context(tc.tile_pool(name="wpool", bufs=1))
        popool = ctx.enter_context(tc.tile_pool(name="popool", bufs=1))

        # ---- tiny consts first (fast DMAs, unblock the gate pipeline) ----
        wzb_sb = const.tile([128, 32], BF16, tag="wzb", name="wzb")
        nc.sync.dma_start(wzb_sb[:], wzb_d[:])
        wzr_sb = const.tile([128, 32], BF16, tag="wzr", name="wzr")
        nc.sync.dma_start(wzr_sb[:], wzr_d[:])
        thr7_sb = const.tile([8, 1], F32, tag="thr7", name="thr7")
        nc.sync.dma_start(thr7_sb[:], thr7_d[:])
        selb_sb = const.tile([7, 7 * 128], BF16, tag="selb", name="selb")
        nc.sync.dma_start(selb_sb[:], selb_d[:])
        cols_sb = const.tile([128, 12], F32, tag="cols", name="cols")
        nc.sync.dma_start(cols_sb[:], cols_d[:])
        sel7_sb = const.tile([7, 8], BF16, tag="sel7", name="sel7")
        nc.sync.dma_start(sel7_sb[:], sel7_d[:])
        cthr_sb = const.tile([8, 1], F32, tag="cthr", name="cthr")
        nc.sync.dma_start(cthr_sb[:], cthr_d[:])
        bw7_sb = const.tile([7, H], BF16, tag="bw7", name="bw7")
        nc.sync.dma_start(bw7_sb[:], bw7_d[:])
        onesb = const.tile([1, 128], BF16, tag="ones", name="ones")
        nc.vector.memset(onesb[:], 1.0)
        onesr = const.tile([1, 128], F32R, tag="onesr", name="onesr")
        nc.vector.memset(onesr[:].bitcast(F32), 1.0)
        identf = const.tile([128, 128], F32, tag="identf", name="identf")
        make_identity(nc, identf[:])
        identb = const.tile([128, 128], BF16, tag="identb", name="identb")
        nc.scalar.activation(identb[:], identf[:], AF.Identity)

        # ---- critical big loads, spread across 4 engine DGE queues so the
        # early DMA bandwidth isn't serialized behind one queue ----
        npool = ctx.enter_context(tc.tile_pool(name="npool", bufs=1))
        hembTb = []
        for ec in range(4):
            t = npool.tile([128, 512], BF16, tag=f"hembTb{ec}", name=f"hembTb{ec}")
            nc.sync.dma_start(t[:], hembtb_d[:, ec * 512:(ec + 1) * 512])
            hembTb.append(t)
        hembRb = []
        for ec in range(4):
            t = npool.tile([128, 512], BF16, tag=f"hembRb{ec}", name=f"hembRb{ec}")
            nc.gpsimd.dma_start(t[:], hembrb_d[:, ec * 512:(ec + 1) * 512])
            hembRb.append(t)
        wpb_sb = []
        for ec in range(4):
            t = npool.tile([128, 512], BF16, tag=f"wpb{ec}", name=f"wpb{ec}")
            nc.scalar.dma_start(t[:], wpb_d[ec * 128:(ec + 1) * 128, :])
            wpb_sb.append(t)
        wcb_sb = []
        for hc in range(4):
            t = npool.tile([128, 1024], BF16, tag=f"wcb{hc}", name=f"wcb{hc}")
            nc.scalar.dma_start(t[:], wcb_d[hc * 128:(hc + 1) * 128, :])
            wcb_sb.append(t)
        wcsb_sb, wab_sb, wbb_sb = [], [], []
        for nm, dt_, lst in (("wcsb", wcsb_d, wcsb_sb), ("wab", wab_d, wab_sb),
                             ("wbb", wbb_d, wbb_sb)):
            for hc in range(4):
                t = npool.tile([128, H], BF16, tag=f"{nm}{hc}", name=f"{nm}{hc}")
                nc.gpsimd.dma_start(t[:], dt_[hc * 128:(hc + 1) * 128, :])
                lst.append(t)

        # pooledT (bf16, [H, pos]) persists across phase A -> B
        pooT = [popool.tile([128, PC_POS], BF16, tag=f"pooT{jc}", name=f"pooT{jc}")
                for jc in range(4)]

        # phase-B weight window stream (issued after phase-A criticals)
        def load_win(w):
            ts_ = []
            for hc in range(4):
                t = wpool.tile([128, VW], BF16, tag=f"ww{hc}",
                               name=f"ww{w}_{hc}", bufs=2)
                nc.sync.dma_start(
                    t[:], woutb_d[hc * 128:(hc + 1) * 128, w * VW:(w + 1) * VW])
                ts_.append(t)
            return ts_

        # ---------------- Phase A ----------------
        with ExitStack() as actx:
            apool = actx.enter_context(tc.tile_pool(name="apool", bufs=1))
            scr = actx.enter_context(tc.tile_pool(name="scr", bufs=2,
                                                  space="PSUM"))

            win_tiles = {0: load_win(0), 1: load_win(1)}

            with nc.allow_low_precision(reason="bf16/f32r matmul inputs"):
                # PE warmup while the input DMAs land: keeps the tensor
                # engine out of its low p-state so phase A runs at full
                # clock.  K=1 matmuls on the memset ones tile are ready
                # almost immediately (no DMA dependency).
                with tc.tile_pool(name="warm", bufs=2, space="PSUM") as wpsum:
                    for wi in range(52):
                        wp_ = wpsum.tile([128, 128], F32, tag="w",
                                         name=f"warm{wi}")
                        nc.tensor.matmul(wp_[:], onesb[:], onesb[:],
                                         start=True, stop=True)
                # gate z rows [8, 512] — all 7 gates in one accumulation.
                # zmask psum pool closes before aggp opens (bank budget).
                masks = []
                with tc.tile_pool(name="zmask", bufs=4, space="PSUM") as zmask:
                    # z = (wzb+wzr)^T (xb + r) to bf16-pair precision:
                    # wzb.xb + wzb.r + wzr.xb  (wzr.r term ~2^-16, dropped)
                    zp = zmask.tile([8, 512], F32, tag="zp", name="zp",
                                    bufs=1)
                    zterms = ([(wzb_sb, hembTb[ec], ec) for ec in range(4)]
                              + [(wzb_sb, hembRb[ec], ec) for ec in range(4)]
                              + [(wzr_sb, hembTb[ec], ec) for ec in range(4)])
                    for i2, (wsb, mov, ec) in enumerate(zterms):
                        nc.tensor.matmul(zp[:], wsb[:, ec * 8:(ec + 1) * 8],
                                         mov[:], start=(i2 == 0),
                                         stop=(i2 == len(zterms) - 1))
                    zr01 = apool.tile([8, 512], BF16, tag="zr01", name="zr01")
                    nc.vector.tensor_scalar(zr01[:], zp[:], thr7_sb[:, 0:1],
                                            None, OP.is_gt)

                    # ancestor-AND + partition broadcast via K=7 matmuls:
                    # psum = (# required gates that fired), mask = psum > n-0.5
                    for m in range(7):
                        ms = zmask.tile([128, 512], F32, tag="mscr",
                                        name=f"ms{m}")
                        nc.tensor.matmul(ms[:],
                                         selb_sb[:, m * 128:(m + 1) * 128],
                                         zr01[0:7, :], start=True, stop=True)
                        mk = apool.tile([128, 512], BF16, tag=f"mask{m}",
                                        name=f"mask{m}")
                        nc.vector.tensor_scalar(mk[:], ms[:],
                                                float(len(MASK_SEL[m])) - 0.5,
                                                None, OP.is_gt)
                        masks.append(mk)
                    # stacked masks [7,512]: all 7 AND-counts as rows, then
                    # one compare with per-row thresholds
                    mr_ps = zmask.tile([8, 512], F32, tag="mr", name="mr",
                                       bufs=1)
                    nc.tensor.matmul(mr_ps[0:7, :], sel7_sb[:, 0:7],
                                     zr01[0:7, :], start=True, stop=True)
                    mrows = apool.tile([7, 512], BF16, tag="mrows",
                                       name="mrows")
                    nc.vector.tensor_scalar(mrows[:], mr_ps[0:7, :],
                                            cthr_sb[0:7, 0:1], None, OP.is_gt)
                e0m, e1Lm, e1Rm, eLLm, eLRm, eRLm, eRRm = masks
                if DEBUG_DUMP:
                    nc.sync.dma_start(dzr_d[:], zr01[:])
                    for m in range(7):
                        nc.sync.dma_start(dmask_d[:, m * 512:(m + 1) * 512],
                                          masks[m][:])

                # h0/n1 psum groups rotate through a 4-bank pool (a 2-bank
                # ring stalls the PE on Act drain latency); opened after
                # zmask closed so the bank budget stays <= 8.
                h0b = []
                n1b = [[None] * 4, [None] * 4]
                with tc.tile_pool(name="scr2", bufs=4, space="PSUM") as scr2:
                    # h0 = embT @ Wp + bp  (bf16 values; gates don't read h0)
                    for hc in range(4):
                        ps = scr2.tile([128, 512], F32, tag="s2",
                                       name=f"h0ps{hc}")
                        for ec in range(4):
                            nc.tensor.matmul(
                                ps[:], wpb_sb[ec][:, hc * 128:(hc + 1) * 128],
                                hembTb[ec][:], start=(ec == 0), stop=(ec == 3))
                        t = apool.tile([128, 512], BF16, tag=f"h0_{hc}",
                                       name=f"h0_{hc}")
                        nc.scalar.activation(t[:], ps[:], AF.Identity,
                                             bias=cols_sb[:, hc:hc + 1])
                        h0b.append(t)
                    if DEBUG_DUMP:
                        for hc in range(4):
                            nc.sync.dma_start(
                                dh0_d[:, hc * 512:(hc + 1) * 512], h0b[hc][:])

                    # level-1 children values (bf16), pair-major so each hc's
                    # (L,R) pair finishes early for the mask products
                    for jc2 in range(4):
                        for side in (0, 1):
                            jq = side * 4 + jc2
                            ps = scr2.tile([128, 512], F32, tag="s2",
                                           name=f"chps{jq}")
                            for hc in range(4):
                                nc.tensor.matmul(
                                    ps[:],
                                    wcb_sb[hc][:, jq * 128:(jq + 1) * 128],
                                    h0b[hc][:], start=(hc == 0), stop=(hc == 3))
                            t = apool.tile([128, 512], BF16,
                                           tag=f"n1_{side}_{jc2}",
                                           name=f"n1_{side}_{jc2}")
                            nc.scalar.activation(t[:], ps[:], AF.Identity,
                                                 bias=cols_sb[:, 4 + jq:5 + jq])
                            n1b[side][jc2] = t

                # count/recip rows first so they don't queue behind the big
                # DVE product chain (masks are ready early)
                esb_t = apool.tile([1, 512], BF16, tag="esb", name="esb")
                nc.vector.tensor_tensor(esb_t[:], masks[0][0:1, :],
                                        masks[1][0:1, :], op=OP.add)
                for m in range(2, 7):
                    nc.vector.tensor_tensor(esb_t[:], esb_t[:],
                                            masks[m][0:1, :], op=OP.add)
                cnt = apool.tile([1, 512], BF16, tag="cnt", name="cnt")
                nc.vector.tensor_scalar(cnt[:], esb_t[:], 2.0, 1.0,
                                        OP.mult, OP.add)
                reci = apool.tile([1, 512], F32, tag="reci", name="reci")
                nc.vector.reciprocal(reci[:], cnt[:])
                recr = apool.tile([1, 512], F32R, tag="recr", name="recr")
                nc.scalar.activation(recr[:], reci[:], AF.Identity)

                # masked sums (bf16 SBUF).  DVE does the n1-gated work; Pool
                # (slow but parallel) gets the h0-gated mults + final adds.
                # mA first so the agg wcs-terms start as early as possible.
                mA, mL, mR = [], [], []
                for hc in range(4):
                    p0 = apool.tile([128, 512], BF16, tag=f"p0{hc}", name=f"p0{hc}")
                    nc.gpsimd.tensor_tensor(p0[:], h0b[hc][:], e0m[:],
                                            op=OP.mult)
                    mA.append(p0)
                for hc in range(4):
                    t1 = apool.tile([128, 512], BF16, tag=f"tA{hc}", name=f"tA{hc}")
                    t1b = apool.tile([128, 512], BF16, tag=f"tB{hc}", name=f"tB{hc}")
                    nc.vector.tensor_tensor(t1[:], n1b[0][hc][:], e1Lm[:],
                                            op=OP.mult)
                    nc.vector.tensor_tensor(t1b[:], n1b[1][hc][:], e1Rm[:],
                                            op=OP.mult)
                    nc.vector.tensor_tensor(t1[:], t1[:], t1b[:], op=OP.add)
                    nc.gpsimd.tensor_tensor(mA[hc][:], mA[hc][:], t1[:],
                                            op=OP.add)
                for hc in range(4):
                    l_ = apool.tile([128, 512], BF16, tag=f"mL{hc}", name=f"mL{hc}")
                    t2 = apool.tile([128, 512], BF16, tag=f"tL{hc}", name=f"tL{hc}")
                    nc.vector.tensor_tensor(l_[:], n1b[0][hc][:], eLLm[:],
                                            op=OP.mult)
                    nc.vector.tensor_tensor(t2[:], n1b[1][hc][:], eRLm[:],
                                            op=OP.mult)
                    nc.vector.tensor_tensor(l_[:], l_[:], t2[:], op=OP.add)
                    mL.append(l_)
                for hc in range(4):
                    r_ = apool.tile([128, 512], BF16, tag=f"mR{hc}", name=f"mR{hc}")
                    t3 = apool.tile([128, 512], BF16, tag=f"tR{hc}", name=f"tR{hc}")
                    nc.vector.tensor_tensor(r_[:], n1b[0][hc][:], eLRm[:],
                                            op=OP.mult)
                    nc.vector.tensor_tensor(t3[:], n1b[1][hc][:], eRRm[:],
                                            op=OP.mult)
                    nc.vector.tensor_tensor(r_[:], r_[:], t3[:], op=OP.add)
                    mR.append(r_)
                if DEBUG_DUMP:
                    for side in (0, 1):
                        for jc2 in range(4):
                            nc.sync.dma_start(
                                dn1_d[:, (side * 4 + jc2) * 512:
                                      (side * 4 + jc2 + 1) * 512],
                                n1b[side][jc2][:])
                    for i, t in enumerate(mA + mL + mR):
                        nc.sync.dma_start(dprod_d[:, i * 512:(i + 1) * 512],
                                          t[:])

                # reciprocal row broadcast to 128 partitions (K=1 f32r)
                rbp = scr.tile([128, 512], F32, tag="s", name="rbp")
                nc.tensor.matmul(rbp[:], onesr[0:1, :], recr[:],
                                 start=True, stop=True)
                rb_sb = apool.tile([128, 512], F32, tag="rb", name="rb")
                nc.scalar.activation(rb_sb[:], rbp[:], AF.Identity)
                if DEBUG_DUMP:
                    nc.sync.dma_start(drow_d[:, 0:512], rb_sb[:])

                # agg accumulation — emitted per-hc so the PE starts on
                # mA[0] while later products are still in flight
                aggp = actx.enter_context(tc.tile_pool(name="aggp", bufs=4,
                                                       space="PSUM"))
                # NOTE: PE accumulation groups must be consecutive — never
                # interleave matmuls of different psum groups.  K=1 matmuls
                # corrupt multi-matmul groups on this toolchain; the bias
                # outer-products go in as a single K=7 matmul instead.
                agg_ps = [aggp.tile([128, 512], F32, tag="agg", name=f"agg{jc}")
                          for jc in range(4)]
                for jc in range(4):
                    ap_ = agg_ps[jc]
                    for hc in range(4):
                        nc.tensor.matmul(ap_[:],
                                         wcsb_sb[hc][:, jc * 128:(jc + 1) * 128],
                                         mA[hc][:], start=(hc == 0), stop=False)
                    for hc in range(4):
                        nc.tensor.matmul(ap_[:],
                                         wab_sb[hc][:, jc * 128:(jc + 1) * 128],
                                         mL[hc][:], start=False, stop=False)
                    for hc in range(4):
                        nc.tensor.matmul(ap_[:],
                                         wbb_sb[hc][:, jc * 128:(jc + 1) * 128],
                                         mR[hc][:], start=False, stop=False)
                    nc.tensor.matmul(ap_[:],
                                     bw7_sb[:, jc * 128:(jc + 1) * 128],
                                     mrows[:], start=False, stop=False)
                    nc.tensor.matmul(ap_[:], identb[:],
                                     h0b[jc][:], start=False, stop=True)
                    if DEBUG_DUMP:
                        dag = apool.tile([128, 512], F32, tag=f"dag{jc}",
                                         name=f"dag{jc}")
                        nc.scalar.activation(dag[:], ap_[:], AF.Identity)
                        nc.sync.dma_start(dagg_d[:, jc * 512:(jc + 1) * 512],
                                          dag[:])
                    nc.vector.tensor_tensor(pooT[jc][:], agg_ps[jc][:],
                                            rb_sb[:], op=OP.mult)
                if DEBUG_DUMP:
                    for jc in range(4):
                        nc.sync.dma_start(dpoo_d[:, jc * 512:(jc + 1) * 512],
                                          pooT[jc][:])

        # ---------------- Phase B ----------------
        with ExitStack() as bctx:
            stp = bctx.enter_context(tc.tile_pool(name="stp", bufs=4))
            mmp = bctx.enter_context(tc.tile_pool(name="mmp", bufs=8,
                                                  space="PSUM"))

            drain_idx = [0]

            def drain(dst_ap, ps_ap):
                # alternate 2:1 DVE:Act — DVE psum->bf16 copies are faster
                if drain_idx[0] % 3 == 2:
                    nc.scalar.activation(dst_ap, ps_ap, AF.Identity)
                else:
                    nc.vector.tensor_copy(dst_ap, ps_ap)
                drain_idx[0] += 1

            with nc.allow_low_precision(reason="bf16 matmul inputs"):
                for w in range(NWIN):
                    wt = win_tiles.pop(w)
                    for pc in range(4):
                        stg = stp.tile([128, VW], BF16, tag="stage",
                                       name=f"stg{w}_{pc}")
                        for s_ in range(NSTR):
                            ps = mmp.tile([128, 500], F32, tag="mm",
                                          name=f"mm{w}_{pc}_{s_}", bufs=8)
                            for hc in range(4):
                                nc.tensor.matmul(
                                    ps[:],
                                    pooT[hc][:, pc * 128:(pc + 1) * 128],
                                    wt[hc][:, s_ * 500:(s_ + 1) * 500],
                                    start=(hc == 0), stop=(hc == 3))
                            drain(stg[:, s_ * 500:(s_ + 1) * 500], ps[:])
                        nc.sync.dma_start(
                            logt_d[pc * 128:(pc + 1) * 128, w * VW:(w + 1) * VW],
                            stg[:])
                    if w + 2 < NWIN:
                        win_tiles[w + 2] = load_win(w + 2)

    nc.compile()
    return nc


def _get_nc():
    if "nc" not in _CACHE:
        _CACHE["nc"] = _build()
    return _CACHE["nc"]


def _prep_inputs(tokens, emb, Wp, bp, Wc, bc, Wg, bg, dep, sib, Wout, bout):
    import ml_dtypes
    BF = ml_dtypes.bfloat16
    f64 = np.float64

    tokens = np.asarray(tokens).astype(np.int64).reshape(-1)
    emb = np.ascontiguousarray(np.asarray(emb, dtype=np.float32))
    Wp = np.asarray(Wp, dtype=f64)
    bp = np.asarray(bp, dtype=f64).reshape(-1)
    Wc = np.asarray(Wc, dtype=f64)
    bc = np.asarray(bc, dtype=f64).reshape(-1)
    Wg = np.asarray(Wg, dtype=f64)
    bg = np.asarray(bg, dtype=f64).reshape(-1)
    dep = np.asarray(dep, dtype=f64)
    sib = np.asarray(sib, dtype=f64)
    Wout = np.asarray(Wout, dtype=np.float32)

    WcL = Wc[:, :H]
    WcR = Wc[:, H:]
    wg = Wg[:, 0]
    wcs = WcL + WcR
    biasL = bc[:H] + SIB_SCALE * sib[0]
    biasR = bc[H:] + SIB_SCALE * sib[1]
    bsum = biasL + biasR

    # folded gate vectors (embedding space) + thresholds
    wgY = {0: WcL @ wg, 1: WcR @ wg}
    zvecs = [Wp @ wg, Wp @ wgY[0], Wp @ wgY[1]]
    thr = [-(bp @ wg + DEPTH_EMBED_SCALE * dep[0] @ wg + bg[0])]
    for X, bX in ((0, biasL), (1, biasR)):
        thr.append(-(bp @ wgY[X] + bX @ wg
                     + DEPTH_EMBED_SCALE * dep[1] @ wg + bg[0]))
    for X, bX in ((0, biasL), (1, biasR)):
        WcX = WcL if X == 0 else WcR
        for Y, bY in ((0, biasL), (1, biasR)):
            v = WcX @ wgY[Y]
            zvecs.append(Wp @ v)
            thr.append(-(bp @ v + bX @ wgY[Y] + bY @ wg
                         + DEPTH_EMBED_SCALE * dep[2] @ wg + bg[0]))
    # zvecs order: [root, L, R, LL, LR, RL, RR]; pad col 7 with zeros
    Wz = np.zeros((E, 8), f64)
    for i, v in enumerate(zvecs):
        Wz[:, i] = v
    wz64 = Wz.reshape(4, 128, 8).transpose(1, 0, 2).reshape(128, 32)
    wzb = np.ascontiguousarray(wz64).astype(np.float32).astype(BF)
    wzr = np.ascontiguousarray(
        wz64 - wzb.astype(f64)).astype(np.float32).astype(BF)
    thr7 = np.zeros((8, 1), f64)
    thr7[:7, 0] = thr
    thr7[7, 0] = 1e30
    thr7 = thr7.astype(np.float32)

    # AND selector columns (0/1), replicated across the 128 out columns
    selb = np.zeros((7, 7 * 128), np.float32)
    for m, sel in enumerate(MASK_SEL):
        for g in sel:
            selb[g, m * 128:(m + 1) * 128] = 1.0
    selb = selb.astype(BF)
    # narrow selector for the stacked-mask-rows matmul, + count thresholds
    sel7 = np.zeros((7, 8), np.float32)
    for m, sel in enumerate(MASK_SEL):
        for g in sel:
            sel7[g, m] = 1.0
    sel7 = sel7.astype(BF)
    cthr = np.full((8, 1), 1e30, np.float32)
    for m, sel in enumerate(MASK_SEL):
        cthr[m, 0] = len(sel) - 0.5

    # per-partition bias columns for act-engine drains (bp, biasL, biasR)
    cols = np.ascontiguousarray(np.concatenate(
        [bp.reshape(4, 128).T, biasL.reshape(4, 128).T, biasR.reshape(4, 128).T],
        axis=1)).astype(np.float32)

    # agg weights (bf16): wcs, WcL@wcs, WcR@wcs; bias rows b3t [3, H]
    wcsb = wcs.astype(np.float32).astype(BF)
    wab = (WcL @ wcs).astype(np.float32).astype(BF)
    wbb = (WcR @ wcs).astype(np.float32).astype(BF)
    # per-mask bias vectors for the K=7 bias matmul:
    # B[j,p] = sum_m bw7[m,j] * mask_m[p]
    cL = biasL @ wcs
    cR = biasR @ wcs
    bw7 = np.zeros((7, H), f64)
    for m in range(7):
        bw7[m] = bsum
        if m in (3, 5):      # eLL, eRL expand a left child
            bw7[m] += cL
        if m in (4, 6):      # eLR, eRR expand a right child
            bw7[m] += cR
    bw7 = bw7.astype(np.float32).astype(BF)

    wpb = np.ascontiguousarray(Wp).astype(np.float32).astype(BF)
    wcb = np.ascontiguousarray(Wc).astype(np.float32).astype(BF)
    woutb = np.ascontiguousarray(Wout.astype(BF))

    in_maps = []
    for c in range(NCORES):
        tk = tokens[c * PC_POS:(c + 1) * PC_POS]
        g = emb[tk]                                   # [512 pos, 512 E] f32
        hembt = np.ascontiguousarray(
            g.T.reshape(4, 128, 512).transpose(1, 0, 2).reshape(128, 4 * 512))
        hembtb = hembt.astype(BF)
        hembrb = (hembt.astype(f64) - hembtb.astype(f64)) \
            .astype(np.float32).astype(BF)
        in_maps.append({
            "wzb": wzb, "wzr": wzr, "thr7": thr7,
            "selb": np.ascontiguousarray(selb),
            "sel7": np.ascontiguousarray(sel7), "cthr": cthr,
            "bw7": np.ascontiguousarray(bw7),
            "cols": cols,
            "hembtb": hembtb, "hembrb": hembrb,
            "wpb": wpb, "wcb": wcb,
            "wcsb": wcsb, "wab": wab, "wbb": wbb,
            "woutb": woutb,
        })
    return in_maps


def _assemble(res, bout=None):
    parts = [np.asarray(res.results[c]["logt"]).astype(np.float32)
             for c in range(NCORES)]
    full = np.concatenate(parts, axis=0)        # [NPOS, V]
    if bout is not None:
        full += np.asarray(bout, dtype=np.float32).reshape(1, V)
    return full.reshape(B, S, V)


def _enable_ldw_opt_once():
    return


def kernel(**inputs) -> np.ndarray:
    from concourse.bass_utils import run_bass_kernel_spmd
    nc = _get_nc()
    in_maps = _prep_inputs(**inputs)
    res = run_bass_kernel_spmd(nc, in_maps, list(range(NCORES)))
    return _assemble(res, bout=inputs["bout"])


# revision 68
# speedup vs baseline: 1.0411x; 1.0411x over previous
"""BoeNet greedy BFS rollout — Trainium2 Bass kernel (8 NeuronCores), v2.

Fully data-parallel: each core owns 512 of the 4096 flattened positions and
computes the full vocab row block for them.  No collectives.

v2 vs v1 (322us baseline):
  * Host ships the gathered embedding rows already TRANSPOSED (hembT
    [E, pos]), both f32 (for gates) and bf16 (for values) — no on-device
    transposes, and phase A starts as soon as the first 256KB chunk lands.
  * All 7 tree gates are folded to embedding space on the host
    (v = Wp Wc... wg), so ONE accumulated matmul (stationary [128,8])
    produces every gate z as rows [8, 512].  Thresholding is one DVE
    tensor_scalar; the ancestor-AND + broadcast to [128,512] is done with
    tiny K=7 matmuls against 0/1 selector columns followed by a compare.
  * h0/n1 values in bf16 (gates no longer read them), halving weight DMA
    and enabling 16-bit DVE throughput for the mask products.
  * Tiny/critical DMAs are issued first; Wout window prefetch is deferred.
  * bout is added on the host: phase-B drains become pure psum->bf16
    copies, alternated between DVE and Act.  16MB of bias-broadcast DMA
    traffic disappears.
  * Phase-B window 0 runs hc-outer so its first matmuls start as soon as
    pooT[0] lands, overlapping the agg tail.
"""
import sys

for _p in ('/opt/trn_rl_repo', '/opt/pypackages'):
    if _p not in sys.path:
        sys.path.insert(0, _p)

import numpy as np

B, S, V, E, H = 8, 512, 32000, 512, 512
NPOS = B * S              # 4096 flattened positions
NCORES = 8
PC_POS = NPOS // NCORES   # 512 positions per core
MAX_DEPTH = 3
DEPTH_EMBED_SCALE = 0.01
SIB_SCALE = 1.0 / np.sqrt(H)

VW = 2000                 # vocab window (4 strips of 500)
NWIN = V // VW            # 16 windows
NSTR = 4                  # 500-col strips per window

# masks: e0, e1L, e1R, e2LL, e2LR, e2RL, e2RR
MASK_SEL = [(0,), (0, 1), (0, 2), (0, 1, 3), (0, 1, 4), (0, 2, 5), (0, 2, 6)]

_CACHE = {}
DEBUG_DUMP = False


def _build():
    import concourse.bass as bass
    import concourse.bacc as bacc
    import concourse.tile as tile
    import concourse.mybir as mybir
    from concourse.masks import make_identity
    from contextlib import ExitStack

    F32 = mybir.dt.float32
    F32R = mybir.dt.float32r
    BF16 = mybir.dt.bfloat16
    AF = mybir.ActivationFunctionType
    OP = mybir.AluOpType

    nc = bacc.Bacc("TRN2", target_bir_lowering=False, debug=False,
                   num_devices=NCORES)

    # --- dram inputs (order here is irrelevant; DMA issue order matters) ---
    wzb_d = nc.dram_tensor("wzb", [128, 32], BF16, kind="ExternalInput")
    wzr_d = nc.dram_tensor("wzr", [128, 32], BF16, kind="ExternalInput")
    thr7_d = nc.dram_tensor("thr7", [8, 1], F32, kind="ExternalInput")
    selb_d = nc.dram_tensor("selb", [7, 7 * 128], BF16, kind="ExternalInput")
    sel7_d = nc.dram_tensor("sel7", [7, 8], BF16, kind="ExternalInput")
    cthr_d = nc.dram_tensor("cthr", [8, 1], F32, kind="ExternalInput")
    bw7_d = nc.dram_tensor("bw7", [7, H], BF16, kind="ExternalInput")
    cols_d = nc.dram_tensor("cols", [128, 12], F32, kind="ExternalInput")
    hembtb_d = nc.dram_tensor("hembtb", [128, 4 * 512], BF16,
                              kind="ExternalInput")
    hembrb_d = nc.dram_tensor("hembrb", [128, 4 * 512], BF16,
                              kind="ExternalInput")
    wpb_d = nc.dram_tensor("wpb", [E, H], BF16, kind="ExternalInput")
    wcb_d = nc.dram_tensor("wcb", [H, 2 * H], BF16, kind="ExternalInput")
    wcsb_d = nc.dram_tensor("wcsb", [H, H], BF16, kind="ExternalInput")
    wab_d = nc.dram_tensor("wab", [H, H], BF16, kind="ExternalInput")
    wbb_d = nc.dram_tensor("wbb", [H, H], BF16, kind="ExternalInput")
    woutb_d = nc.dram_tensor("woutb", [H, V], BF16, kind="ExternalInput")
    logt_d = nc.dram_tensor("logt", [PC_POS, V], BF16, kind="ExternalOutput")
    if DEBUG_DUMP:
        dzr_d = nc.dram_tensor("dzr", [8, 512], BF16, kind="ExternalOutput")
        dmask_d = nc.dram_tensor("dmask", [128, 7 * 512], BF16,
                                 kind="ExternalOutput")
        dh0_d = nc.dram_tensor("dh0", [128, 4 * 512], BF16,
                               kind="ExternalOutput")
        dpoo_d = nc.dram_tensor("dpoo", [128, 4 * 512], BF16,
                                kind="ExternalOutput")
        dn1_d = nc.dram_tensor("dn1", [128, 8 * 512], BF16,
                               kind="ExternalOutput")
        dprod_d = nc.dram_tensor("dprod", [128, 12 * 512], BF16,
                                 kind="ExternalOutput")
        drow_d = nc.dram_tensor("drow", [128, 4 * 512], mybir.dt.float32,
                                kind="ExternalOutput")
        dagg_d = nc.dram_tensor("dagg", [128, 4 * 512], mybir.dt.float32,
                                kind="ExternalOutput")

    with tile.TileContext(nc) as tc, ExitStack() as ctx:
        const = ctx.enter_context(tc.tile_pool(name="const", bufs=1))
        wpool = ctx.enter_context(tc.tile_pool(name="wpool", bufs=1))
        popool = ctx.enter_context(tc.tile_pool(name="popool", bufs=1))

        # ---- tiny consts first (fast DMAs, unblock the gate pipeline) ----
        wzb_sb = const.tile([128, 32], BF16, tag="wzb", name="wzb")
        nc.sync.dma_start(wzb_sb[:], wzb_d[:])
        wzr_sb = const.tile([128, 32], BF16, tag="wzr", name="wzr")
        nc.sync.dma_start(wzr_sb[:], wzr_d[:])
        thr7_sb = const.tile([8, 1], F32, tag="thr7", name="thr7")
        nc.sync.dma_start(thr7_sb[:], thr7_d[:])
        selb_sb = const.tile([7, 7 * 128], BF16, tag="selb", name="selb")
        nc.sync.dma_start(selb_sb[:], selb_d[:])
        cols_sb = const.tile([128, 12], F32, tag="cols", name="cols")
        nc.sync.dma_start(cols_sb[:], cols_d[:])
        sel7_sb = const.tile([7, 8], BF16, tag="sel7", name="sel7")
        nc.sync.dma_start(sel7_sb[:], sel7_d[:])
        cthr_sb = const.tile([8, 1], F32, tag="cthr", name="cthr")
        nc.sync.dma_start(cthr_sb[:], cthr_d[:])
        bw7_sb = const.tile([7, H], BF16, tag="bw7", name="bw7")
        nc.sync.dma_start(bw7_sb[:], bw7_d[:])
        onesb = const.tile([1, 128], BF16, tag="ones", name="ones")
        nc.vector.memset(onesb[:], 1.0)
        onesr = const.tile([1, 128], F32R, tag="onesr", name="onesr")
        nc.vector.memset(onesr[:].bitcast(F32), 1.0)
        identf = const.tile([128, 128], F32, tag="identf", name="identf")
        make_identity(nc, identf[:])
        identb = const.tile([128, 128], BF16, tag="identb", name="identb")
        nc.scalar.activation(identb[:], identf[:], AF.Identity)

        # ---- critical big loads, spread across 4 engine DGE queues so the
        # early DMA bandwidth isn't serialized behind one queue ----
        npool = ctx.enter_context(tc.tile_pool(name="npool", bufs=1))
        hembTb = []
        for ec in range(4):
            t = npool.tile([128, 512], BF16, tag=f"hembTb{ec}", name=f"hembTb{ec}")
            nc.scalar.dma_start(t[:], hembtb_d[:, ec * 512:(ec + 1) * 512])
            hembTb.append(t)
        hembRb = []
        for ec in range(4):
            t = npool.tile([128, 512], BF16, tag=f"hembRb{ec}", name=f"hembRb{ec}")
            nc.gpsimd.dma_start(t[:], hembrb_d[:, ec * 512:(ec + 1) * 512])
            hembRb.append(t)
        wpb_sb = []
        for ec in range(4):
            t = npool.tile([128, 512], BF16, tag=f"wpb{ec}", name=f"wpb{ec}")
            nc.sync.dma_start(t[:], wpb_d[ec * 128:(ec + 1) * 128, :])
            wpb_sb.append(t)
        wcb_sb = []
        for hc in range(4):
            t = npool.tile([128, 1024], BF16, tag=f"wcb{hc}", name=f"wcb{hc}")
            nc.sync.dma_start(t[:], wcb_d[hc * 128:(hc + 1) * 128, :])
            wcb_sb.append(t)
        wcsb_sb, wab_sb, wbb_sb = [], [], []
        for nm, dt_, lst in (("wcsb", wcsb_d, wcsb_sb), ("wab", wab_d, wab_sb),
                             ("wbb", wbb_d, wbb_sb)):
            for hc in range(4):
                t = npool.tile([128, H], BF16, tag=f"{nm}{hc}", name=f"{nm}{hc}")
                nc.gpsimd.dma_start(t[:], dt_[hc * 128:(hc + 1) * 128, :])
                lst.append(t)

        # pooledT (bf16, [H, pos]) persists across phase A -> B
        pooT = [popool.tile([128, PC_POS], BF16, tag=f"pooT{jc}", name=f"pooT{jc}")
                for jc in range(4)]

        # phase-B weight window stream (issued after phase-A criticals)
        def load_win(w):
            ts_ = []
            for hc in range(4):
                t = wpool.tile([128, VW], BF16, tag=f"ww{hc}",
                               name=f"ww{w}_{hc}", bufs=2)
                nc.sync.dma_start(
                    t[:], woutb_d[hc * 128:(hc + 1) * 128, w * VW:(w + 1) * VW])
                ts_.append(t)
            return ts_

        # ---------------- Phase A ----------------
        with ExitStack() as actx:
            apool = actx.enter_context(tc.tile_pool(name="apool", bufs=1))
            scr = actx.enter_context(tc.tile_pool(name="scr", bufs=2,
                                                  space="PSUM"))

            win_tiles = {0: load_win(0), 1: load_win(1)}

            with nc.allow_low_precision(reason="bf16/f32r matmul inputs"):
                # PE warmup while the input DMAs land: keeps the tensor
                # engine out of its low p-state so phase A runs at full
                # clock.  K=1 matmuls on the memset ones tile are ready
                # almost immediately (no DMA dependency).
                with tc.tile_pool(name="warm", bufs=2, space="PSUM") as wpsum:
                    for wi in range(52):
                        wp_ = wpsum.tile([128, 128], F32, tag="w",
                                         name=f"warm{wi}")
                        nc.tensor.matmul(wp_[:], onesb[:], onesb[:],
                                         start=True, stop=True)
                # gate z rows [8, 512] — all 7 gates in one accumulation.
                # zmask psum pool closes before aggp opens (bank budget).
                masks = []
                with tc.tile_pool(name="zmask", bufs=4, space="PSUM") as zmask:
                    # z = (wzb+wzr)^T (xb + r) to bf16-pair precision:
                    # wzb.xb + wzb.r + wzr.xb  (wzr.r term ~2^-16, dropped)
                    zp = zmask.tile([8, 512], F32, tag="zp", name="zp",
                                    bufs=1)
                    zterms = ([(wzb_sb, hembTb[ec], ec) for ec in range(4)]
                              + [(wzb_sb, hembRb[ec], ec) for ec in range(4)]
                              + [(wzr_sb, hembTb[ec], ec) for ec in range(4)])
                    for i2, (wsb, mov, ec) in enumerate(zterms):
                        nc.tensor.matmul(zp[:], wsb[:, ec * 8:(ec + 1) * 8],
                                         mov[:], start=(i2 == 0),
                                         stop=(i2 == len(zterms) - 1))
                    zr01 = apool.tile([8, 512], BF16, tag="zr01", name="zr01")
                    nc.vector.tensor_scalar(zr01[:], zp[:], thr7_sb[:, 0:1],
                                            None, OP.is_gt)

                    # ancestor-AND + partition broadcast via K=7 matmuls:
                    # psum = (# required gates that fired), mask = psum > n-0.5
                    for m in range(7):
                        ms = zmask.tile([128, 512], F32, tag="mscr",
                                        name=f"ms{m}")
                        nc.tensor.matmul(ms[:],
                                         selb_sb[:, m * 128:(m + 1) * 128],
                                         zr01[0:7, :], start=True, stop=True)
                        mk = apool.tile([128, 512], BF16, tag=f"mask{m}",
                                        name=f"mask{m}")
                        nc.vector.tensor_scalar(mk[:], ms[:],
                                                float(len(MASK_SEL[m])) - 0.5,
                                                None, OP.is_gt)
                        masks.append(mk)
                    # stacked masks [7,512]: all 7 AND-counts as rows, then
                    # one compare with per-row thresholds
                    mr_ps = zmask.tile([8, 512], F32, tag="mr", name="mr",
                                       bufs=1)
                    nc.tensor.matmul(mr_ps[0:7, :], sel7_sb[:, 0:7],
                                     zr01[0:7, :], start=True, stop=True)
                    mrows = apool.tile([7, 512], BF16, tag="mrows",
                                       name="mrows")
                    nc.vector.tensor_scalar(mrows[:], mr_ps[0:7, :],
                                            cthr_sb[0:7, 0:1], None, OP.is_gt)
                e0m, e1Lm, e1Rm, eLLm, eLRm, eRLm, eRRm = masks
                if DEBUG_DUMP:
                    nc.sync.dma_start(dzr_d[:], zr01[:])
                    for m in range(7):
                        nc.sync.dma_start(dmask_d[:, m * 512:(m + 1) * 512],
                                          masks[m][:])

                # h0/n1 psum groups rotate through a 4-bank pool (a 2-bank
                # ring stalls the PE on Act drain latency); opened after
                # zmask closed so the bank budget stays <= 8.
                h0b = []
                n1b = [[None] * 4, [None] * 4]
                with tc.tile_pool(name="scr2", bufs=4, space="PSUM") as scr2:
                    # h0 = embT @ Wp + bp  (bf16 values; gates don't read h0)
                    for hc in range(4):
                        ps = scr2.tile([128, 512], F32, tag="s2",
                                       name=f"h0ps{hc}")
                        for ec in range(4):
                            nc.tensor.matmul(
                                ps[:], wpb_sb[ec][:, hc * 128:(hc + 1) * 128],
                                hembTb[ec][:], start=(ec == 0), stop=(ec == 3))
                        t = apool.tile([128, 512], BF16, tag=f"h0_{hc}",
                                       name=f"h0_{hc}")
                        nc.scalar.activation(t[:], ps[:], AF.Identity,
                                             bias=cols_sb[:, hc:hc + 1])
                        h0b.append(t)
                    if DEBUG_DUMP:
                        for hc in range(4):
                            nc.sync.dma_start(
                                dh0_d[:, hc * 512:(hc + 1) * 512], h0b[hc][:])

                    # level-1 children values (bf16), pair-major so each hc's
                    # (L,R) pair finishes early for the mask products
                    for jc2 in range(4):
                        for side in (0, 1):
                            jq = side * 4 + jc2
                            ps = scr2.tile([128, 512], F32, tag="s2",
                                           name=f"chps{jq}")
                            for hc in range(4):
                                nc.tensor.matmul(
                                    ps[:],
                                    wcb_sb[hc][:, jq * 128:(jq + 1) * 128],
                                    h0b[hc][:], start=(hc == 0), stop=(hc == 3))
                            t = apool.tile([128, 512], BF16,
                                           tag=f"n1_{side}_{jc2}",
                                           name=f"n1_{side}_{jc2}")
                            nc.scalar.activation(t[:], ps[:], AF.Identity,
                                                 bias=cols_sb[:, 4 + jq:5 + jq])
                            n1b[side][jc2] = t

                # count/recip rows first so they don't queue behind the big
                # DVE product chain (masks are ready early)
                esb_t = apool.tile([1, 512], BF16, tag="esb", name="esb")
                nc.vector.tensor_tensor(esb_t[:], masks[0][0:1, :],
                                        masks[1][0:1, :], op=OP.add)
                for m in range(2, 7):
                    nc.vector.tensor_tensor(esb_t[:], esb_t[:],
                                            masks[m][0:1, :], op=OP.add)
                cnt = apool.tile([1, 512], BF16, tag="cnt", name="cnt")
                nc.vector.tensor_scalar(cnt[:], esb_t[:], 2.0, 1.0,
                                        OP.mult, OP.add)
                reci = apool.tile([1, 512], F32, tag="reci", name="reci")
                nc.vector.reciprocal(reci[:], cnt[:])
                recr = apool.tile([1, 512], F32R, tag="recr", name="recr")
                nc.scalar.activation(recr[:], reci[:], AF.Identity)

                # masked sums (bf16 SBUF).  DVE does the n1-gated work; Pool
                # (slow but parallel) gets the h0-gated mults + final adds.
                # mA first so the agg wcs-terms start as early as possible.
                mA, mL, mR = [], [], []
                for hc in range(4):
                    p0 = apool.tile([128, 512], BF16, tag=f"p0{hc}", name=f"p0{hc}")
                    nc.gpsimd.tensor_tensor(p0[:], h0b[hc][:], e0m[:],
                                            op=OP.mult)
                    mA.append(p0)
                for hc in range(4):
                    t1 = apool.tile([128, 512], BF16, tag=f"tA{hc}", name=f"tA{hc}")
                    t1b = apool.tile([128, 512], BF16, tag=f"tB{hc}", name=f"tB{hc}")
                    nc.vector.tensor_tensor(t1[:], n1b[0][hc][:], e1Lm[:],
                                            op=OP.mult)
                    nc.vector.tensor_tensor(t1b[:], n1b[1][hc][:], e1Rm[:],
                                            op=OP.mult)
                    nc.vector.tensor_tensor(t1[:], t1[:], t1b[:], op=OP.add)
                    nc.gpsimd.tensor_tensor(mA[hc][:], mA[hc][:], t1[:],
                                            op=OP.add)
                for hc in range(4):
                    l_ = apool.tile([128, 512], BF16, tag=f"mL{hc}", name=f"mL{hc}")
                    t2 = apool.tile([128, 512], BF16, tag=f"tL{hc}", name=f"tL{hc}")
                    nc.vector.tensor_tensor(l_[:], n1b[0][hc][:], eLLm[:],
                                            op=OP.mult)
                    nc.vector.tensor_tensor(t2[:], n1b[1][hc][:], eRLm[:],
                                            op=OP.mult)
                    nc.vector.tensor_tensor(l_[:], l_[:], t2[:], op=OP.add)
                    mL.append(l_)
                for hc in range(4):
                    r_ = apool.tile([128, 512], BF16, tag=f"mR{hc}", name=f"mR{hc}")
                    t3 = apool.tile([128, 512], BF16, tag=f"tR{hc}", name=f"tR{hc}")
                    nc.vector.tensor_tensor(r_[:], n1b[0][hc][:], eLRm[:],
                                            op=OP.mult)
                    nc.vector.tensor_tensor(t3[:], n1b[1][hc][:], eRRm[:],
                                            op=OP.mult)
                    nc.vector.tensor_tensor(r_[:], r_[:], t3[:], op=OP.add)
                    mR.append(r_)
                if DEBUG_DUMP:
                    for side in (0, 1):
                        for jc2 in range(4):
                            nc.sync.dma_start(
                                dn1_d[:, (side * 4 + jc2) * 512:
                                      (side * 4 + jc2 + 1) * 512],
                                n1b[side][jc2][:])
                    for i, t in enumerate(mA + mL + mR):
                        nc.sync.dma_start(dprod_d[:, i * 512:(i + 1) * 512],
                                          t[:])

                # reciprocal row broadcast to 128 partitions (K=1 f32r)
                rbp = scr.tile([128, 512], F32, tag="s", name="rbp")
                nc.tensor.matmul(rbp[:], onesr[0:1, :], recr[:],
                                 start=True, stop=True)
                rb_sb = apool.tile([128, 512], F32, tag="rb", name="rb")
                nc.scalar.activation(rb_sb[:], rbp[:], AF.Identity)
                if DEBUG_DUMP:
                    nc.sync.dma_start(drow_d[:, 0:512], rb_sb[:])

                # agg accumulation — emitted per-hc so the PE starts on
                # mA[0] while later products are still in flight
                aggp = actx.enter_context(tc.tile_pool(name="aggp", bufs=4,
                                                       space="PSUM"))
                # NOTE: PE accumulation groups must be consecutive — never
                # interleave matmuls of different psum groups.  K=1 matmuls
                # corrupt multi-matmul groups on this toolchain; the bias
                # outer-products go in as a single K=7 matmul instead.
                agg_ps = [aggp.tile([128, 512], F32, tag="agg", name=f"agg{jc}")
                          for jc in range(4)]
                for jc in range(4):
                    ap_ = agg_ps[jc]
                    for hc in range(4):
                        nc.tensor.matmul(ap_[:],
                                         wcsb_sb[hc][:, jc * 128:(jc + 1) * 128],
                                         mA[hc][:], start=(hc == 0), stop=False)
                    for hc in range(4):
                        nc.tensor.matmul(ap_[:],
                                         wab_sb[hc][:, jc * 128:(jc + 1) * 128],
                                         mL[hc][:], start=False, stop=False)
                    for hc in range(4):
                        nc.tensor.matmul(ap_[:],
                                         wbb_sb[hc][:, jc * 128:(jc + 1) * 128],
                                         mR[hc][:], start=False, stop=False)
                    nc.tensor.matmul(ap_[:],
                                     bw7_sb[:, jc * 128:(jc + 1) * 128],
                                     mrows[:], start=False, stop=False)
                    nc.tensor.matmul(ap_[:], identb[:],
                                     h0b[jc][:], start=False, stop=True)
                    if DEBUG_DUMP:
                        dag = apool.tile([128, 512], F32, tag=f"dag{jc}",
                                         name=f"dag{jc}")
                        nc.scalar.activation(dag[:], ap_[:], AF.Identity)
                        nc.sync.dma_start(dagg_d[:, jc * 512:(jc + 1) * 512],
                                          dag[:])
                    nc.vector.tensor_tensor(pooT[jc][:], agg_ps[jc][:],
                                            rb_sb[:], op=OP.mult)
                if DEBUG_DUMP:
                    for jc in range(4):
                        nc.sync.dma_start(dpoo_d[:, jc * 512:(jc + 1) * 512],
                                          pooT[jc][:])

        # ---------------- Phase B ----------------
        with ExitStack() as bctx:
            stp = bctx.enter_context(tc.tile_pool(name="stp", bufs=4))
            mmp = bctx.enter_context(tc.tile_pool(name="mmp", bufs=8,
                                                  space="PSUM"))

            drain_idx = [0]

            def drain(dst_ap, ps_ap):
                # alternate 2:1 DVE:Act — DVE psum->bf16 copies are faster
                if drain_idx[0] % 3 == 2:
                    nc.scalar.activation(dst_ap, ps_ap, AF.Identity)
                else:
                    nc.vector.tensor_copy(dst_ap, ps_ap)
                drain_idx[0] += 1

            with nc.allow_low_precision(reason="bf16 matmul inputs"):
                for w in range(NWIN):
                    wt = win_tiles.pop(w)
                    for pc in range(4):
                        stg = stp.tile([128, VW], BF16, tag="stage",
                                       name=f"stg{w}_{pc}")
                        for s_ in range(NSTR):
                            ps = mmp.tile([128, 500], F32, tag="mm",
                                          name=f"mm{w}_{pc}_{s_}", bufs=8)
                            for hc in range(4):
                                nc.tensor.matmul(
                                    ps[:],
                                    pooT[hc][:, pc * 128:(pc + 1) * 128],
                                    wt[hc][:, s_ * 500:(s_ + 1) * 500],
                                    start=(hc == 0), stop=(hc == 3))
                            drain(stg[:, s_ * 500:(s_ + 1) * 500], ps[:])
                        nc.sync.dma_start(
                            logt_d[pc * 128:(pc + 1) * 128, w * VW:(w + 1) * VW],
                            stg[:])
                    if w + 2 < NWIN:
                        win_tiles[w + 2] = load_win(w + 2)

    nc.compile()
    return nc


def _get_nc():
    if "nc" not in _CACHE:
        _CACHE["nc"] = _build()
    return _CACHE["nc"]


def _prep_inputs(tokens, emb, Wp, bp, Wc, bc, Wg, bg, dep, sib, Wout, bout):
    import ml_dtypes
    BF = ml_dtypes.bfloat16
    f64 = np.float64

    tokens = np.asarray(tokens).astype(np.int64).reshape(-1)
    emb = np.ascontiguousarray(np.asarray(emb, dtype=np.float32))
    Wp = np.asarray(Wp, dtype=f64)
    bp = np.asarray(bp, dtype=f64).reshape(-1)
    Wc = np.asarray(Wc, dtype=f64)
    bc = np.asarray(bc, dtype=f64).reshape(-1)
    Wg = np.asarray(Wg, dtype=f64)
    bg = np.asarray(bg, dtype=f64).reshape(-1)
    dep = np.asarray(dep, dtype=f64)
    sib = np.asarray(sib, dtype=f64)
    Wout = np.asarray(Wout, dtype=np.float32)

    WcL = Wc[:, :H]
    WcR = Wc[:, H:]
    wg = Wg[:, 0]
    wcs = WcL + WcR
    biasL = bc[:H] + SIB_SCALE * sib[0]
    biasR = bc[H:] + SIB_SCALE * sib[1]
    bsum = biasL + biasR

    # folded gate vectors (embedding space) + thresholds
    wgY = {0: WcL @ wg, 1: WcR @ wg}
    zvecs = [Wp @ wg, Wp @ wgY[0], Wp @ wgY[1]]
    thr = [-(bp @ wg + DEPTH_EMBED_SCALE * dep[0] @ wg + bg[0])]
    for X, bX in ((0, biasL), (1, biasR)):
        thr.append(-(bp @ wgY[X] + bX @ wg
                     + DEPTH_EMBED_SCALE * dep[1] @ wg + bg[0]))
    for X, bX in ((0, biasL), (1, biasR)):
        WcX = WcL if X == 0 else WcR
        for Y, bY in ((0, biasL), (1, biasR)):
            v = WcX @ wgY[Y]
            zvecs.append(Wp @ v)
            thr.append(-(bp @ v + bX @ wgY[Y] + bY @ wg
                         + DEPTH_EMBED_SCALE * dep[2] @ wg + bg[0]))
    # zvecs order: [root, L, R, LL, LR, RL, RR]; pad col 7 with zeros
    Wz = np.zeros((E, 8), f64)
    for i, v in enumerate(zvecs):
        Wz[:, i] = v
    wz64 = Wz.reshape(4, 128, 8).transpose(1, 0, 2).reshape(128, 32)
    wzb = np.ascontiguousarray(wz64).astype(np.float32).astype(BF)
    wzr = np.ascontiguousarray(
        wz64 - wzb.astype(f64)).astype(np.float32).astype(BF)
    thr7 = np.zeros((8, 1), f64)
    thr7[:7, 0] = thr
    thr7[7, 0] = 1e30
    thr7 = thr7.astype(np.float32)

    # AND selector columns (0/1), replicated across the 128 out columns
    selb = np.zeros((7, 7 * 128), np.float32)
    for m, sel in enumerate(MASK_SEL):
        for g in sel:
            selb[g, m * 128:(m + 1) * 128] = 1.0
    selb = selb.astype(BF)
    # narrow selector for the stacked-mask-rows matmul, + count thresholds
    sel7 = np.zeros((7, 8), np.float32)
    for m, sel in enumerate(MASK_SEL):
        for g in sel:
            sel7[g, m] = 1.0
    sel7 = sel7.astype(BF)
    cthr = np.full((8, 1), 1e30, np.float32)
    for m, sel in enumerate(MASK_SEL):
        cthr[m, 0] = len(sel) - 0.5

    # per-partition bias columns for act-engine drains (bp, biasL, biasR)
    cols = np.ascontiguousarray(np.concatenate(
        [bp.reshape(4, 128).T, biasL.reshape(4, 128).T, biasR.reshape(4, 128).T],
        axis=1)).astype(np.float32)

    # agg weights (bf16): wcs, WcL@wcs, WcR@wcs; bias rows b3t [3, H]
    wcsb = wcs.astype(np.float32).astype(BF)
    wab = (WcL @ wcs).astype(np.float32).astype(BF)
    wbb = (WcR @ wcs).astype(np.float32).astype(BF)
    # per-mask bias vectors for the K=7 bias matmul:
    # B[j,p] = sum_m bw7[m,j] * mask_m[p]
    cL = biasL @ wcs
    cR = biasR @ wcs
    bw7 = np.zeros((7, H), f64)
    for m in range(7):
        bw7[m] = bsum
        if m in (3, 5):      # eLL, eRL expand a left child
            bw7[m] += cL
        if m in (4, 6):      # eLR, eRR expand a right child
            bw7[m] += cR
    bw7 = bw7.astype(np.float32).astype(BF)

    wpb = np.ascontiguousarray(Wp).astype(np.float32).astype(BF)
    wcb = np.ascontiguousarray(Wc).astype(np.float32).astype(BF)
    woutb = np.ascontiguousarray(Wout.astype(BF))

    in_maps = []
    for c in range(NCORES):
        tk = tokens[c * PC_POS:(c + 1) * PC_POS]
        g = emb[tk]                                   # [512 pos, 512 E] f32
        hembt = np.ascontiguousarray(
            g.T.reshape(4, 128, 512).transpose(1, 0, 2).reshape(128, 4 * 512))
        hembtb = hembt.astype(BF)
        hembrb = (hembt.astype(f64) - hembtb.astype(f64)) \
            .astype(np.float32).astype(BF)
        in_maps.append({
            "wzb": wzb, "wzr": wzr, "thr7": thr7,
            "selb": np.ascontiguousarray(selb),
            "sel7": np.ascontiguousarray(sel7), "cthr": cthr,
            "bw7": np.ascontiguousarray(bw7),
            "cols": cols,
            "hembtb": hembtb, "hembrb": hembrb,
            "wpb": wpb, "wcb": wcb,
            "wcsb": wcsb, "wab": wab, "wbb": wbb,
            "woutb": woutb,
        })
    return in_maps


def _assemble(res, bout=None):
    parts = [np.asarray(res.results[c]["logt"]).astype(np.float32)
             for c in range(NCORES)]
    full = np.concatenate(parts, axis=0)        # [NPOS, V]
    if bout is not None:
        full += np.asarray(bout, dtype=np.float32).reshape(1, V)
    return full.reshape(B, S, V)


def _enable_ldw_opt_once():
    return


def kernel(**inputs) -> np.ndarray:
    from concourse.bass_utils import run_bass_kernel_spmd
    nc = _get_nc()
    in_maps = _prep_inputs(**inputs)
    res = run_bass_kernel_spmd(nc, in_maps, list(range(NCORES)))
    return _assemble(res, bout=inputs["bout"])


# revision 69
# speedup vs baseline: 1.0446x; 1.0033x over previous
"""BoeNet greedy BFS rollout — Trainium2 Bass kernel (8 NeuronCores), v2.

Fully data-parallel: each core owns 512 of the 4096 flattened positions and
computes the full vocab row block for them.  No collectives.

v2 vs v1 (322us baseline):
  * Host ships the gathered embedding rows already TRANSPOSED (hembT
    [E, pos]), both f32 (for gates) and bf16 (for values) — no on-device
    transposes, and phase A starts as soon as the first 256KB chunk lands.
  * All 7 tree gates are folded to embedding space on the host
    (v = Wp Wc... wg), so ONE accumulated matmul (stationary [128,8])
    produces every gate z as rows [8, 512].  Thresholding is one DVE
    tensor_scalar; the ancestor-AND + broadcast to [128,512] is done with
    tiny K=7 matmuls against 0/1 selector columns followed by a compare.
  * h0/n1 values in bf16 (gates no longer read them), halving weight DMA
    and enabling 16-bit DVE throughput for the mask products.
  * Tiny/critical DMAs are issued first; Wout window prefetch is deferred.
  * bout is added on the host: phase-B drains become pure psum->bf16
    copies, alternated between DVE and Act.  16MB of bias-broadcast DMA
    traffic disappears.
  * Phase-B window 0 runs hc-outer so its first matmuls start as soon as
    pooT[0] lands, overlapping the agg tail.
"""
import sys

for _p in ('/opt/trn_rl_repo', '/opt/pypackages'):
    if _p not in sys.path:
        sys.path.insert(0, _p)

import numpy as np

B, S, V, E, H = 8, 512, 32000, 512, 512
NPOS = B * S              # 4096 flattened positions
NCORES = 8
PC_POS = NPOS // NCORES   # 512 positions per core
MAX_DEPTH = 3
DEPTH_EMBED_SCALE = 0.01
SIB_SCALE = 1.0 / np.sqrt(H)

VW = 2000                 # vocab window (4 strips of 500)
NWIN = V // VW            # 16 windows
NSTR = 4                  # 500-col strips per window

# masks: e0, e1L, e1R, e2LL, e2LR, e2RL, e2RR
MASK_SEL = [(0,), (0, 1), (0, 2), (0, 1, 3), (0, 1, 4), (0, 2, 5), (0, 2, 6)]

_CACHE = {}
DEBUG_DUMP = False


def _build():
    import concourse.bass as bass
    import concourse.bacc as bacc
    import concourse.tile as tile
    import concourse.mybir as mybir
    from concourse.masks import make_identity
    from contextlib import ExitStack

    F32 = mybir.dt.float32
    F32R = mybir.dt.float32r
    BF16 = mybir.dt.bfloat16
    AF = mybir.ActivationFunctionType
    OP = mybir.AluOpType

    nc = bacc.Bacc("TRN2", target_bir_lowering=False, debug=False,
                   num_devices=NCORES)

    # --- dram inputs (order here is irrelevant; DMA issue order matters) ---
    wzb_d = nc.dram_tensor("wzb", [128, 32], BF16, kind="ExternalInput")
    wzr_d = nc.dram_tensor("wzr", [128, 32], BF16, kind="ExternalInput")
    thr7_d = nc.dram_tensor("thr7", [8, 1], F32, kind="ExternalInput")
    selb_d = nc.dram_tensor("selb", [7, 7 * 128], BF16, kind="ExternalInput")
    sel7_d = nc.dram_tensor("sel7", [7, 8], BF16, kind="ExternalInput")
    cthr_d = nc.dram_tensor("cthr", [8, 1], F32, kind="ExternalInput")
    bw7_d = nc.dram_tensor("bw7", [7, H], BF16, kind="ExternalInput")
    cols_d = nc.dram_tensor("cols", [128, 12], F32, kind="ExternalInput")
    hembtb_d = nc.dram_tensor("hembtb", [128, 4 * 512], BF16,
                              kind="ExternalInput")
    hembrb_d = nc.dram_tensor("hembrb", [128, 4 * 512], BF16,
                              kind="ExternalInput")
    wpb_d = nc.dram_tensor("wpb", [E, H], BF16, kind="ExternalInput")
    wcb_d = nc.dram_tensor("wcb", [H, 2 * H], BF16, kind="ExternalInput")
    wcsb_d = nc.dram_tensor("wcsb", [H, H], BF16, kind="ExternalInput")
    wab_d = nc.dram_tensor("wab", [H, H], BF16, kind="ExternalInput")
    wbb_d = nc.dram_tensor("wbb", [H, H], BF16, kind="ExternalInput")
    woutb_d = nc.dram_tensor("woutb", [H, V], BF16, kind="ExternalInput")
    logt_d = nc.dram_tensor("logt", [PC_POS, V], BF16, kind="ExternalOutput")
    if DEBUG_DUMP:
        dzr_d = nc.dram_tensor("dzr", [8, 512], BF16, kind="ExternalOutput")
        dmask_d = nc.dram_tensor("dmask", [128, 7 * 512], BF16,
                                 kind="ExternalOutput")
        dh0_d = nc.dram_tensor("dh0", [128, 4 * 512], BF16,
                               kind="ExternalOutput")
        dpoo_d = nc.dram_tensor("dpoo", [128, 4 * 512], BF16,
                                kind="ExternalOutput")
        dn1_d = nc.dram_tensor("dn1", [128, 8 * 512], BF16,
                               kind="ExternalOutput")
        dprod_d = nc.dram_tensor("dprod", [128, 12 * 512], BF16,
                                 kind="ExternalOutput")
        drow_d = nc.dram_tensor("drow", [128, 4 * 512], mybir.dt.float32,
                                kind="ExternalOutput")
        dagg_d = nc.dram_tensor("dagg", [128, 4 * 512], mybir.dt.float32,
                                kind="ExternalOutput")

    with tile.TileContext(nc) as tc, ExitStack() as ctx:
        const = ctx.enter_context(tc.tile_pool(name="const", bufs=1))
        wpool = ctx.enter_context(tc.tile_pool(name="wpool", bufs=1))
        popool = ctx.enter_context(tc.tile_pool(name="popool", bufs=1))

        # ---- tiny consts first (fast DMAs, unblock the gate pipeline) ----
        wzb_sb = const.tile([128, 32], BF16, tag="wzb", name="wzb")
        nc.sync.dma_start(wzb_sb[:], wzb_d[:])
        wzr_sb = const.tile([128, 32], BF16, tag="wzr", name="wzr")
        nc.sync.dma_start(wzr_sb[:], wzr_d[:])
        thr7_sb = const.tile([8, 1], F32, tag="thr7", name="thr7")
        nc.sync.dma_start(thr7_sb[:], thr7_d[:])
        selb_sb = const.tile([7, 7 * 128], BF16, tag="selb", name="selb")
        nc.sync.dma_start(selb_sb[:], selb_d[:])
        cols_sb = const.tile([128, 12], F32, tag="cols", name="cols")
        nc.sync.dma_start(cols_sb[:], cols_d[:])
        sel7_sb = const.tile([7, 8], BF16, tag="sel7", name="sel7")
        nc.sync.dma_start(sel7_sb[:], sel7_d[:])
        cthr_sb = const.tile([8, 1], F32, tag="cthr", name="cthr")
        nc.sync.dma_start(cthr_sb[:], cthr_d[:])
        bw7_sb = const.tile([7, H], BF16, tag="bw7", name="bw7")
        nc.sync.dma_start(bw7_sb[:], bw7_d[:])
        onesb = const.tile([1, 128], BF16, tag="ones", name="ones")
        nc.vector.memset(onesb[:], 1.0)
        onesr = const.tile([1, 128], F32R, tag="onesr", name="onesr")
        nc.vector.memset(onesr[:].bitcast(F32), 1.0)
        identf = const.tile([128, 128], F32, tag="identf", name="identf")
        make_identity(nc, identf[:])
        identb = const.tile([128, 128], BF16, tag="identb", name="identb")
        nc.scalar.activation(identb[:], identf[:], AF.Identity)

        # ---- critical big loads, spread across 4 engine DGE queues so the
        # early DMA bandwidth isn't serialized behind one queue ----
        npool = ctx.enter_context(tc.tile_pool(name="npool", bufs=1))
        hembTb = []
        for ec in range(4):
            t = npool.tile([128, 512], BF16, tag=f"hembTb{ec}", name=f"hembTb{ec}")
            nc.scalar.dma_start(t[:], hembtb_d[:, ec * 512:(ec + 1) * 512])
            hembTb.append(t)
        hembRb = []
        for ec in range(4):
            t = npool.tile([128, 512], BF16, tag=f"hembRb{ec}", name=f"hembRb{ec}")
            nc.gpsimd.dma_start(t[:], hembrb_d[:, ec * 512:(ec + 1) * 512])
            hembRb.append(t)
        wpb_sb = []
        for ec in range(4):
            t = npool.tile([128, 512], BF16, tag=f"wpb{ec}", name=f"wpb{ec}")
            nc.scalar.dma_start(t[:], wpb_d[ec * 128:(ec + 1) * 128, :])
            wpb_sb.append(t)
        wcb_sb = []
        for hc in range(4):
            t = npool.tile([128, 1024], BF16, tag=f"wcb{hc}", name=f"wcb{hc}")
            nc.scalar.dma_start(t[:], wcb_d[hc * 128:(hc + 1) * 128, :])
            wcb_sb.append(t)
        wcsb_sb, wab_sb, wbb_sb = [], [], []
        for nm, dt_, lst in (("wcsb", wcsb_d, wcsb_sb), ("wab", wab_d, wab_sb),
                             ("wbb", wbb_d, wbb_sb)):
            for hc in range(4):
                t = npool.tile([128, H], BF16, tag=f"{nm}{hc}", name=f"{nm}{hc}")
                nc.gpsimd.dma_start(t[:], dt_[hc * 128:(hc + 1) * 128, :])
                lst.append(t)

        # pooledT (bf16, [H, pos]) persists across phase A -> B
        pooT = [popool.tile([128, PC_POS], BF16, tag=f"pooT{jc}", name=f"pooT{jc}")
                for jc in range(4)]

        # phase-B weight window stream (issued after phase-A criticals)
        def load_win(w):
            ts_ = []
            for hc in range(4):
                t = wpool.tile([128, VW], BF16, tag=f"ww{hc}",
                               name=f"ww{w}_{hc}", bufs=2)
                nc.sync.dma_start(
                    t[:], woutb_d[hc * 128:(hc + 1) * 128, w * VW:(w + 1) * VW])
                ts_.append(t)
            return ts_

        # ---------------- Phase A ----------------
        with ExitStack() as actx:
            apool = actx.enter_context(tc.tile_pool(name="apool", bufs=1))
            scr = actx.enter_context(tc.tile_pool(name="scr", bufs=2,
                                                  space="PSUM"))

            win_tiles = {0: load_win(0), 1: load_win(1)}

            with nc.allow_low_precision(reason="bf16/f32r matmul inputs"):
                # PE warmup while the input DMAs land: keeps the tensor
                # engine out of its low p-state so phase A runs at full
                # clock.  K=1 matmuls on the memset ones tile are ready
                # almost immediately (no DMA dependency).
                with tc.tile_pool(name="warm", bufs=2, space="PSUM") as wpsum:
                    for wi in range(52):
                        wp_ = wpsum.tile([128, 128], F32, tag="w",
                                         name=f"warm{wi}")
                        nc.tensor.matmul(wp_[:], onesb[:], onesb[:],
                                         start=True, stop=True)
                # gate z rows [8, 512] — all 7 gates in one accumulation.
                # zmask psum pool closes before aggp opens (bank budget).
                masks = []
                with tc.tile_pool(name="zmask", bufs=4, space="PSUM") as zmask:
                    # z = (wzb+wzr)^T (xb + r) to bf16-pair precision:
                    # wzb.xb + wzb.r + wzr.xb  (wzr.r term ~2^-16, dropped)
                    zp = zmask.tile([8, 512], F32, tag="zp", name="zp",
                                    bufs=1)
                    zterms = ([(wzb_sb, hembTb[ec], ec) for ec in range(4)]
                              + [(wzb_sb, hembRb[ec], ec) for ec in range(4)]
                              + [(wzr_sb, hembTb[ec], ec) for ec in range(4)])
                    for i2, (wsb, mov, ec) in enumerate(zterms):
                        nc.tensor.matmul(zp[:], wsb[:, ec * 8:(ec + 1) * 8],
                                         mov[:], start=(i2 == 0),
                                         stop=(i2 == len(zterms) - 1))
                    zr01 = apool.tile([8, 512], BF16, tag="zr01", name="zr01")
                    nc.vector.tensor_scalar(zr01[:], zp[:], thr7_sb[:, 0:1],
                                            None, OP.is_gt)

                    # ancestor-AND + partition broadcast via K=7 matmuls:
                    # psum = (# required gates that fired), mask = psum > n-0.5
                    for m in range(7):
                        ms = zmask.tile([128, 512], F32, tag="mscr",
                                        name=f"ms{m}")
                        nc.tensor.matmul(ms[:],
                                         selb_sb[:, m * 128:(m + 1) * 128],
                                         zr01[0:7, :], start=True, stop=True)
                        mk = apool.tile([128, 512], BF16, tag=f"mask{m}",
                                        name=f"mask{m}")
                        nc.vector.tensor_scalar(mk[:], ms[:],
                                                float(len(MASK_SEL[m])) - 0.5,
                                                None, OP.is_gt)
                        masks.append(mk)
                    # stacked masks [7,512]: all 7 AND-counts as rows, then
                    # one compare with per-row thresholds
                    mr_ps = zmask.tile([8, 512], F32, tag="mr", name="mr",
                                       bufs=1)
                    nc.tensor.matmul(mr_ps[0:7, :], sel7_sb[:, 0:7],
                                     zr01[0:7, :], start=True, stop=True)
                    mrows = apool.tile([7, 512], BF16, tag="mrows",
                                       name="mrows")
                    nc.vector.tensor_scalar(mrows[:], mr_ps[0:7, :],
                                            cthr_sb[0:7, 0:1], None, OP.is_gt)
                e0m, e1Lm, e1Rm, eLLm, eLRm, eRLm, eRRm = masks
                if DEBUG_DUMP:
                    nc.sync.dma_start(dzr_d[:], zr01[:])
                    for m in range(7):
                        nc.sync.dma_start(dmask_d[:, m * 512:(m + 1) * 512],
                                          masks[m][:])

                # h0/n1 psum groups rotate through a 4-bank pool (a 2-bank
                # ring stalls the PE on Act drain latency); opened after
                # zmask closed so the bank budget stays <= 8.
                h0b = []
                n1b = [[None] * 4, [None] * 4]
                with tc.tile_pool(name="scr2", bufs=4, space="PSUM") as scr2:
                    # h0 = embT @ Wp + bp  (bf16 values; gates don't read h0)
                    for hc in range(4):
                        ps = scr2.tile([128, 512], F32, tag="s2",
                                       name=f"h0ps{hc}")
                        for ec in range(4):
                            nc.tensor.matmul(
                                ps[:], wpb_sb[ec][:, hc * 128:(hc + 1) * 128],
                                hembTb[ec][:], start=(ec == 0), stop=(ec == 3))
                        t = apool.tile([128, 512], BF16, tag=f"h0_{hc}",
                                       name=f"h0_{hc}")
                        nc.scalar.activation(t[:], ps[:], AF.Identity,
                                             bias=cols_sb[:, hc:hc + 1])
                        h0b.append(t)
                    if DEBUG_DUMP:
                        for hc in range(4):
                            nc.sync.dma_start(
                                dh0_d[:, hc * 512:(hc + 1) * 512], h0b[hc][:])

                    # level-1 children values (bf16), pair-major so each hc's
                    # (L,R) pair finishes early for the mask products
                    for jc2 in range(4):
                        for side in (0, 1):
                            jq = side * 4 + jc2
                            ps = scr2.tile([128, 512], F32, tag="s2",
                                           name=f"chps{jq}")
                            for hc in range(4):
                                nc.tensor.matmul(
                                    ps[:],
                                    wcb_sb[hc][:, jq * 128:(jq + 1) * 128],
                                    h0b[hc][:], start=(hc == 0), stop=(hc == 3))
                            t = apool.tile([128, 512], BF16,
                                           tag=f"n1_{side}_{jc2}",
                                           name=f"n1_{side}_{jc2}")
                            nc.scalar.activation(t[:], ps[:], AF.Identity,
                                                 bias=cols_sb[:, 4 + jq:5 + jq])
                            n1b[side][jc2] = t

                # count/recip rows first so they don't queue behind the big
                # DVE product chain (masks are ready early)
                esb_t = apool.tile([1, 512], BF16, tag="esb", name="esb")
                nc.vector.tensor_tensor(esb_t[:], masks[0][0:1, :],
                                        masks[1][0:1, :], op=OP.add)
                for m in range(2, 7):
                    nc.vector.tensor_tensor(esb_t[:], esb_t[:],
                                            masks[m][0:1, :], op=OP.add)
                cnt = apool.tile([1, 512], BF16, tag="cnt", name="cnt")
                nc.vector.tensor_scalar(cnt[:], esb_t[:], 2.0, 1.0,
                                        OP.mult, OP.add)
                reci = apool.tile([1, 512], F32, tag="reci", name="reci")
                nc.vector.reciprocal(reci[:], cnt[:])
                recr = apool.tile([1, 512], F32R, tag="recr", name="recr")
                nc.scalar.activation(recr[:], reci[:], AF.Identity)

                # masked sums (bf16 SBUF).  DVE does the n1-gated work; Pool
                # (slow but parallel) gets the h0-gated mults + final adds.
                # mA first so the agg wcs-terms start as early as possible.
                mA, mL, mR = [], [], []
                for hc in range(4):
                    p0 = apool.tile([128, 512], BF16, tag=f"p0{hc}", name=f"p0{hc}")
                    nc.gpsimd.tensor_tensor(p0[:], h0b[hc][:], e0m[:],
                                            op=OP.mult)
                    mA.append(p0)
                for hc in range(4):
                    t1 = apool.tile([128, 512], BF16, tag=f"tA{hc}", name=f"tA{hc}")
                    t1b = apool.tile([128, 512], BF16, tag=f"tB{hc}", name=f"tB{hc}")
                    nc.vector.tensor_tensor(t1[:], n1b[0][hc][:], e1Lm[:],
                                            op=OP.mult)
                    nc.vector.tensor_tensor(t1b[:], n1b[1][hc][:], e1Rm[:],
                                            op=OP.mult)
                    nc.vector.tensor_tensor(t1[:], t1[:], t1b[:], op=OP.add)
                    nc.gpsimd.tensor_tensor(mA[hc][:], mA[hc][:], t1[:],
                                            op=OP.add)
                for hc in range(4):
                    l_ = apool.tile([128, 512], BF16, tag=f"mL{hc}", name=f"mL{hc}")
                    t2 = apool.tile([128, 512], BF16, tag=f"tL{hc}", name=f"tL{hc}")
                    nc.vector.tensor_tensor(l_[:], n1b[0][hc][:], eLLm[:],
                                            op=OP.mult)
                    nc.vector.tensor_tensor(t2[:], n1b[1][hc][:], eRLm[:],
                                            op=OP.mult)
                    nc.vector.tensor_tensor(l_[:], l_[:], t2[:], op=OP.add)
                    mL.append(l_)
                for hc in range(4):
                    r_ = apool.tile([128, 512], BF16, tag=f"mR{hc}", name=f"mR{hc}")
                    t3 = apool.tile([128, 512], BF16, tag=f"tR{hc}", name=f"tR{hc}")
                    nc.vector.tensor_tensor(r_[:], n1b[0][hc][:], eLRm[:],
                                            op=OP.mult)
                    nc.vector.tensor_tensor(t3[:], n1b[1][hc][:], eRRm[:],
                                            op=OP.mult)
                    nc.vector.tensor_tensor(r_[:], r_[:], t3[:], op=OP.add)
                    mR.append(r_)
                if DEBUG_DUMP:
                    for side in (0, 1):
                        for jc2 in range(4):
                            nc.sync.dma_start(
                                dn1_d[:, (side * 4 + jc2) * 512:
                                      (side * 4 + jc2 + 1) * 512],
                                n1b[side][jc2][:])
                    for i, t in enumerate(mA + mL + mR):
                        nc.sync.dma_start(dprod_d[:, i * 512:(i + 1) * 512],
                                          t[:])

                # reciprocal row broadcast to 128 partitions (K=1 f32r)
                rbp = scr.tile([128, 512], F32, tag="s", name="rbp")
                nc.tensor.matmul(rbp[:], onesr[0:1, :], recr[:],
                                 start=True, stop=True)
                rb_sb = apool.tile([128, 512], F32, tag="rb", name="rb")
                nc.scalar.activation(rb_sb[:], rbp[:], AF.Identity)
                if DEBUG_DUMP:
                    nc.sync.dma_start(drow_d[:, 0:512], rb_sb[:])

                # agg accumulation — emitted per-hc so the PE starts on
                # mA[0] while later products are still in flight
                aggp = actx.enter_context(tc.tile_pool(name="aggp", bufs=4,
                                                       space="PSUM"))
                # NOTE: PE accumulation groups must be consecutive — never
                # interleave matmuls of different psum groups.  K=1 matmuls
                # corrupt multi-matmul groups on this toolchain; the bias
                # outer-products go in as a single K=7 matmul instead.
                agg_ps = [aggp.tile([128, 512], F32, tag="agg", name=f"agg{jc}")
                          for jc in range(4)]
                for jc in range(4):
                    ap_ = agg_ps[jc]
                    for hc in range(4):
                        nc.tensor.matmul(ap_[:],
                                         wcsb_sb[hc][:, jc * 128:(jc + 1) * 128],
                                         mA[hc][:], start=(hc == 0), stop=False)
                    for hc in range(4):
                        nc.tensor.matmul(ap_[:],
                                         wab_sb[hc][:, jc * 128:(jc + 1) * 128],
                                         mL[hc][:], start=False, stop=False)
                    for hc in range(4):
                        nc.tensor.matmul(ap_[:],
                                         wbb_sb[hc][:, jc * 128:(jc + 1) * 128],
                                         mR[hc][:], start=False, stop=False)
                    nc.tensor.matmul(ap_[:],
                                     bw7_sb[:, jc * 128:(jc + 1) * 128],
                                     mrows[:], start=False, stop=False)
                    nc.tensor.matmul(ap_[:], identb[:],
                                     h0b[jc][:], start=False, stop=True)
                    if DEBUG_DUMP:
                        dag = apool.tile([128, 512], F32, tag=f"dag{jc}",
                                         name=f"dag{jc}")
                        nc.scalar.activation(dag[:], ap_[:], AF.Identity)
                        nc.sync.dma_start(dagg_d[:, jc * 512:(jc + 1) * 512],
                                          dag[:])
                    nc.vector.tensor_tensor(pooT[jc][:], agg_ps[jc][:],
                                            rb_sb[:], op=OP.mult)
                if DEBUG_DUMP:
                    for jc in range(4):
                        nc.sync.dma_start(dpoo_d[:, jc * 512:(jc + 1) * 512],
                                          pooT[jc][:])

        # ---------------- Phase B ----------------
        with ExitStack() as bctx:
            stp = bctx.enter_context(tc.tile_pool(name="stp", bufs=4))
            mmp = bctx.enter_context(tc.tile_pool(name="mmp", bufs=8,
                                                  space="PSUM"))

            drain_idx = [0]

            def drain(dst_ap, ps_ap):
                # alternate 2:1 DVE:Act — DVE psum->bf16 copies are faster
                if drain_idx[0] % 3 == 2:
                    nc.scalar.activation(dst_ap, ps_ap, AF.Identity)
                else:
                    nc.vector.tensor_copy(dst_ap, ps_ap)
                drain_idx[0] += 1

            with nc.allow_low_precision(reason="bf16 matmul inputs"):
                for w in range(NWIN):
                    wt = win_tiles.pop(w)
                    for pc in range(4):
                        stg = stp.tile([128, VW], BF16, tag="stage",
                                       name=f"stg{w}_{pc}")
                        for s_ in range(NSTR):
                            ps = mmp.tile([128, 500], F32, tag="mm",
                                          name=f"mm{w}_{pc}_{s_}", bufs=8)
                            for hc in range(4):
                                nc.tensor.matmul(
                                    ps[:],
                                    pooT[hc][:, pc * 128:(pc + 1) * 128],
                                    wt[hc][:, s_ * 500:(s_ + 1) * 500],
                                    start=(hc == 0), stop=(hc == 3))
                            drain(stg[:, s_ * 500:(s_ + 1) * 500], ps[:])
                        nc.sync.dma_start(
                            logt_d[pc * 128:(pc + 1) * 128, w * VW:(w + 1) * VW],
                            stg[:])
                    if w + 2 < NWIN:
                        win_tiles[w + 2] = load_win(w + 2)

    nc.compile()
    return nc


def _get_nc():
    if "nc" not in _CACHE:
        _CACHE["nc"] = _build()
    return _CACHE["nc"]


def _prep_inputs(tokens, emb, Wp, bp, Wc, bc, Wg, bg, dep, sib, Wout, bout):
    import ml_dtypes
    BF = ml_dtypes.bfloat16
    f64 = np.float64

    tokens = np.asarray(tokens).astype(np.int64).reshape(-1)
    emb = np.ascontiguousarray(np.asarray(emb, dtype=np.float32))
    Wp = np.asarray(Wp, dtype=f64)
    bp = np.asarray(bp, dtype=f64).reshape(-1)
    Wc = np.asarray(Wc, dtype=f64)
    bc = np.asarray(bc, dtype=f64).reshape(-1)
    Wg = np.asarray(Wg, dtype=f64)
    bg = np.asarray(bg, dtype=f64).reshape(-1)
    dep = np.asarray(dep, dtype=f64)
    sib = np.asarray(sib, dtype=f64)
    Wout = np.asarray(Wout, dtype=np.float32)

    WcL = Wc[:, :H]
    WcR = Wc[:, H:]
    wg = Wg[:, 0]
    wcs = WcL + WcR
    biasL = bc[:H] + SIB_SCALE * sib[0]
    biasR = bc[H:] + SIB_SCALE * sib[1]
    bsum = biasL + biasR

    # folded gate vectors (embedding space) + thresholds
    wgY = {0: WcL @ wg, 1: WcR @ wg}
    zvecs = [Wp @ wg, Wp @ wgY[0], Wp @ wgY[1]]
    thr = [-(bp @ wg + DEPTH_EMBED_SCALE * dep[0] @ wg + bg[0])]
    for X, bX in ((0, biasL), (1, biasR)):
        thr.append(-(bp @ wgY[X] + bX @ wg
                     + DEPTH_EMBED_SCALE * dep[1] @ wg + bg[0]))
    for X, bX in ((0, biasL), (1, biasR)):
        WcX = WcL if X == 0 else WcR
        for Y, bY in ((0, biasL), (1, biasR)):
            v = WcX @ wgY[Y]
            zvecs.append(Wp @ v)
            thr.append(-(bp @ v + bX @ wgY[Y] + bY @ wg
                         + DEPTH_EMBED_SCALE * dep[2] @ wg + bg[0]))
    # zvecs order: [root, L, R, LL, LR, RL, RR]; pad col 7 with zeros
    Wz = np.zeros((E, 8), f64)
    for i, v in enumerate(zvecs):
        Wz[:, i] = v
    wz64 = Wz.reshape(4, 128, 8).transpose(1, 0, 2).reshape(128, 32)
    wzb = np.ascontiguousarray(wz64).astype(np.float32).astype(BF)
    wzr = np.ascontiguousarray(
        wz64 - wzb.astype(f64)).astype(np.float32).astype(BF)
    thr7 = np.zeros((8, 1), f64)
    thr7[:7, 0] = thr
    thr7[7, 0] = 1e30
    thr7 = thr7.astype(np.float32)

    # AND selector columns (0/1), replicated across the 128 out columns
    selb = np.zeros((7, 7 * 128), np.float32)
    for m, sel in enumerate(MASK_SEL):
        for g in sel:
            selb[g, m * 128:(m + 1) * 128] = 1.0
    selb = selb.astype(BF)
    # narrow selector for the stacked-mask-rows matmul, + count thresholds
    sel7 = np.zeros((7, 8), np.float32)
    for m, sel in enumerate(MASK_SEL):
        for g in sel:
            sel7[g, m] = 1.0
    sel7 = sel7.astype(BF)
    cthr = np.full((8, 1), 1e30, np.float32)
    for m, sel in enumerate(MASK_SEL):
        cthr[m, 0] = len(sel) - 0.5

    # per-partition bias columns for act-engine drains (bp, biasL, biasR)
    cols = np.ascontiguousarray(np.concatenate(
        [bp.reshape(4, 128).T, biasL.reshape(4, 128).T, biasR.reshape(4, 128).T],
        axis=1)).astype(np.float32)

    # agg weights (bf16): wcs, WcL@wcs, WcR@wcs; bias rows b3t [3, H]
    wcsb = wcs.astype(np.float32).astype(BF)
    wab = (WcL @ wcs).astype(np.float32).astype(BF)
    wbb = (WcR @ wcs).astype(np.float32).astype(BF)
    # per-mask bias vectors for the K=7 bias matmul:
    # B[j,p] = sum_m bw7[m,j] * mask_m[p]
    cL = biasL @ wcs
    cR = biasR @ wcs
    bw7 = np.zeros((7, H), f64)
    for m in range(7):
        bw7[m] = bsum
        if m in (3, 5):      # eLL, eRL expand a left child
            bw7[m] += cL
        if m in (4, 6):      # eLR, eRR expand a right child
            bw7[m] += cR
    bw7 = bw7.astype(np.float32).astype(BF)

    wpb = np.ascontiguousarray(Wp).astype(np.float32).astype(BF)
    wcb = np.ascontiguousarray(Wc).astype(np.float32).astype(BF)
    woutb = np.ascontiguousarray(Wout.astype(BF))

    in_maps = []
    for c in range(NCORES):
        tk = tokens[c * PC_POS:(c + 1) * PC_POS]
        g = emb[tk]                                   # [512 pos, 512 E] f32
        hembt = np.ascontiguousarray(
            g.T.reshape(4, 128, 512).transpose(1, 0, 2).reshape(128, 4 * 512))
        hembtb = hembt.astype(BF)
        hembrb = (hembt.astype(f64) - hembtb.astype(f64)) \
            .astype(np.float32).astype(BF)
        in_maps.append({
            "wzb": wzb, "wzr": wzr, "thr7": thr7,
            "selb": np.ascontiguousarray(selb),
            "sel7": np.ascontiguousarray(sel7), "cthr": cthr,
            "bw7": np.ascontiguousarray(bw7),
            "cols": cols,
            "hembtb": hembtb, "hembrb": hembrb,
            "wpb": wpb, "wcb": wcb,
            "wcsb": wcsb, "wab": wab, "wbb": wbb,
            "woutb": woutb,
        })
    return in_maps


def _assemble(res, bout=None):
    parts = [np.asarray(res.results[c]["logt"]).astype(np.float32)
             for c in range(NCORES)]
    full = np.concatenate(parts, axis=0)        # [NPOS, V]
    if bout is not None:
        full += np.asarray(bout, dtype=np.float32).reshape(1, V)
    return full.reshape(B, S, V)


def _enable_ldw_opt_once():
    return


def kernel(**inputs) -> np.ndarray:
    from concourse.bass_utils import run_bass_kernel_spmd
    nc = _get_nc()
    in_maps = _prep_inputs(**inputs)
    res = run_bass_kernel_spmd(nc, in_maps, list(range(NCORES)))
    return _assemble(res, bout=inputs["bout"])
